# revision 5
# baseline (speedup 1.0000x reference)
"""RegionFusionNetwork Trainium2 kernel.

Strategy (8 NeuronCores, SPMD single program):
- ROI-Align is expressed per ROI as pooled[c,bin] = sum_pix region[c,pix] *
  (Ay (x) Ax)[pix,bin]  -- a Kronecker-factored interpolation matrix applied
  with the tensor engine after an on-chip region transpose.
- The whole post-pooling network is linear, so the three 6272->512 FCs, the
  fusion FC and both heads collapse on the host into one [26, 6272] matrix per
  feature map; the device GEMM contracts pooled features directly to the
  26 output columns (cls 2 + reg 24).
- Per-ROI bbox regions are DMA'd channels-first [c=128, Hc, Wc] with
  register-driven dynamic base offsets; shapes are made static by bucketing
  ROIs into size classes, with identical class counts on every core (host
  deals ROIs round-robin within each class, padding with dummy slots).
- Data-parallel over the 2000 proposals; feature maps + weights replicated.
"""
import numpy as np

OUT = 7
SR = 2
P14 = OUT * SR
C = 128
N_CORES = 8
MAPS = {"bev": (800, 704), "fv": (64, 512), "rgb": (128, 512)}
MAP_NAMES = ["bev", "fv", "rgb"]
FEAT_KEY = {"bev": "feat_bev", "fv": "feat_fv", "rgb": "feat_rgb"}
GROUP = 8          # slots per A-build batch
GEMM_M = 64        # slots per GEMM output batch
DT_POOL = "f16"    # pooling-path dtype: "f16" or "f32"
WSCALE = 1024.0    # head-weight scale (undone on host) to avoid fp16 subnormals

LAST_EXEC_TIME_NS = None  # set by kernel() when a HW profile is available
LAST_RUN_WALL_NS = None   # wall time of the execute call (incl. transfers)

_GRID = [2, 3, 4, 5, 6, 7, 8, 9, 10, 11, 12, 13, 14, 15, 16, 17, 18, 19, 20,
         21, 22, 23, 24, 26, 28, 30, 33, 36, 39, 42, 46, 50, 55, 60, 66, 72,
         79, 87, 95, 104]


def _f32(x):
    return np.asarray(x, dtype=np.float32)


def _grid_up(v, cap):
    for g in _GRID:
        if g >= v:
            return min(g, cap)
    return cap


def _project_bev(p):
    b = p[:, 0]
    x_img = (p[:, 1] - np.float32(0.0)) / np.float32(0.1)
    y_img = (p[:, 2] - np.float32(-40.0)) / np.float32(0.1)
    l_img = p[:, 4] / np.float32(0.1)
    w_img = p[:, 5] / np.float32(0.1)
    hl = l_img / np.float32(2)
    hw = w_img / np.float32(2)
    return np.stack([b, x_img - hl, y_img - hw, x_img + hl, y_img + hw],
                    axis=1).astype(np.float32)


def _sample_geometry(rois, H, W):
    x1, y1, x2, y2 = rois[:, 1], rois[:, 2], rois[:, 3], rois[:, 4]
    roi_w = np.maximum(x2 - x1, np.float32(1.0))
    roi_h = np.maximum(y2 - y1, np.float32(1.0))
    frac = (np.arange(P14, dtype=np.float32) + np.float32(0.5)) / np.float32(SR)
    ty = (roi_h / np.float32(OUT))[:, None]
    tx = (roi_w / np.float32(OUT))[:, None]
    gy = y1[:, None] + frac[None, :] * ty
    gx = x1[:, None] + frac[None, :] * tx
    vy = (gy >= np.float32(-1.0)) & (gy <= np.float32(H))
    vx = (gx >= np.float32(-1.0)) & (gx <= np.float32(W))
    y = np.clip(gy, np.float32(0.0), np.float32(H - 1))
    x = np.clip(gx, np.float32(0.0), np.float32(W - 1))
    y0 = np.floor(y).astype(np.int32)
    x0 = np.floor(x).astype(np.int32)
    y1i = np.minimum(y0 + 1, H - 1)
    x1i = np.minimum(x0 + 1, W - 1)
    ly = y - y0.astype(np.float32)
    lx = x - x0.astype(np.float32)
    return dict(y0=y0, y1i=y1i, x0=x0, x1i=x1i, ly=ly, lx=lx, vy=vy, vx=vx,
                by0=y0.min(1), by1=y1i.max(1), bx0=x0.min(1), bx1=x1i.max(1))


def _axis_factor(idx0, idx1, lo, valid, origin, size):
    """A-axis factor [size, 7]: 0.5*(1-lo) at idx0, 0.5*lo at idx1 per sample."""
    A = np.zeros((size, OUT), np.float32)
    half = np.float32(0.5)
    for p in range(P14):
        if not valid[p]:
            continue
        i = p // SR
        w1 = lo[p]
        A[idx0[p] - origin, i] += half * (np.float32(1.0) - w1)
        A[idx1[p] - origin, i] += half * w1
    return A


def _plan_map(rois, H, W, bidx):
    """Per-map plan: classes, per-core slots, per-ROI origin + factors."""
    N = rois.shape[0]
    g = _sample_geometry(rois, H, W)
    Hr = g["by1"] - g["by0"] + 1
    Wr = g["bx1"] - g["bx0"] + 1
    Hc = np.array([_grid_up(int(h), H) for h in Hr], np.int32)
    Wc = np.array([_grid_up(int(w), W) for w in Wr], np.int32)
    oy = np.minimum(g["by0"], H - Hc)
    ox = np.minimum(g["bx0"], W - Wc)
    cls_of = {}
    for r in range(N):
        cls_of.setdefault((int(Hc[r]), int(Wc[r])), []).append(r)
    classes = sorted(cls_of.keys())
    # per-core slot lists, identical structure across cores
    core_slots = [[] for _ in range(N_CORES)]
    slot_classes = []   # (Hc, Wc, nch, count_per_core) per class block
    for key in classes:
        rlist = cls_of[key]
        per_core = (len(rlist) + N_CORES - 1) // N_CORES
        nch = (key[0] * key[1] + 127) // 128
        slot_classes.append((key[0], key[1], nch, per_core))
        for i in range(per_core * N_CORES):
            core_slots[i % N_CORES].append(rlist[i] if i < len(rlist) else -1)
    S = len(core_slots[0])
    return dict(g=g, oy=oy, ox=ox, Hc=Hc, Wc=Wc, classes=slot_classes,
                core_slots=core_slots, S=S, H=H, W=W, bidx=bidx)


def _build_plan(inputs):
    prop = _f32(inputs["proposals3d"])
    bidx = prop[:, 0].astype(np.int32)
    rois = {
        "bev": _project_bev(prop),
        "fv": np.concatenate([prop[:, 0:1],
                              _f32(inputs["rand_fv"]) * np.float32(50.0)],
                             axis=1).astype(np.float32),
        "rgb": np.concatenate([prop[:, 0:1],
                               _f32(inputs["rand_rgb"]) * np.float32(100.0)],
                              axis=1).astype(np.float32),
    }
    plan = {}
    for m in MAP_NAMES:
        H, W = MAPS[m]
        plan[m] = _plan_map(rois[m], H, W, bidx)
    return plan


def _pack_core(plan, core):
    """meta offsets [TOT], afac [128, TOTCH*14] for one core."""
    np_dt = np.float16 if DT_POOL == "f16" else np.float32
    metas = []
    afac_cols = []
    for m in MAP_NAMES:
        pm = plan[m]
        H, W = pm["H"], pm["W"]
        g = pm["g"]
        slots = pm["core_slots"][core]
        si = 0
        for (hc, wc, nch, count) in pm["classes"]:
            for _ in range(count):
                r = slots[si]
                si += 1
                blk = np.zeros((128, nch * 14), np.float32)
                if r < 0:
                    metas.append(0)
                else:
                    o_y, o_x = int(pm["oy"][r]), int(pm["ox"][r])
                    metas.append(int(pm["bidx"][r]) * C * H * W + o_y * W + o_x)
                    Ay = _axis_factor(g["y0"][r], g["y1i"][r], g["ly"][r],
                                      g["vy"][r], o_y, hc)
                    Ax = _axis_factor(g["x0"][r], g["x1i"][r], g["lx"][r],
                                      g["vx"][r], o_x, wc)
                    npix = hc * wc
                    t = np.arange(nch * 128)
                    ok = t < npix
                    ayr = np.zeros((nch * 128, OUT), np.float32)
                    axr = np.zeros((nch * 128, OUT), np.float32)
                    ayr[ok] = Ay[t[ok] // wc]
                    axr[ok] = Ax[t[ok] % wc]
                    blk[:, :] = np.concatenate(
                        [ayr.reshape(nch, 128, OUT), axr.reshape(nch, 128, OUT)],
                        axis=2).transpose(1, 0, 2).reshape(128, nch * 14)
                afac_cols.append(blk)
    afac = np.concatenate(afac_cols, axis=1).astype(np_dt)
    return np.asarray(metas, np.int32).reshape(1, -1), afac


def _collapse_head(inputs):
    W_fus = np.float64(1.0) * inputs["W_fus"]
    mats = {}
    for m, wk in (("bev", "W_bev"), ("fv", "W_fv"), ("rgb", "W_rgb")):
        core = np.concatenate([inputs["W_cls"], inputs["W_reg"]], axis=0)
        M = (core.astype(np.float64) @ W_fus.astype(np.float64)
             @ inputs[wk].astype(np.float64)) / 3.0
        mats[m] = M  # [26, 6272]
    b_bar = (inputs["b_bev"] + inputs["b_fv"] + inputs["b_rgb"]).astype(np.float64) / 3.0
    t = inputs["b_fus"].astype(np.float64) + inputs["W_fus"].astype(np.float64) @ b_bar
    cst = np.concatenate([
        inputs["b_cls"].astype(np.float64) + inputs["W_cls"].astype(np.float64) @ t,
        inputs["b_reg"].astype(np.float64) + inputs["W_reg"].astype(np.float64) @ t,
    ])
    return mats, cst


def _pack_wtil(mats):
    """wtil [128, 3*49*26]: wtil[c, (m*49+bin)*26+o] = M_m[o, c*49+bin]*WSCALE."""
    np_dt = np.float16 if DT_POOL == "f16" else np.float32
    scale = WSCALE if DT_POOL == "f16" else 1.0
    w = np.zeros((128, 3 * 49 * 26), np.float64)
    for mi, m in enumerate(MAP_NAMES):
        M = mats[m].reshape(26, 128, 49)  # [o, c, bin]
        w[:, (mi * 49) * 26:(mi * 49 + 49) * 26] = (
            M.transpose(1, 2, 0).reshape(128, 49 * 26) * scale)
    return w.astype(np_dt)


def _trace_program(plan, tot_slots, totch):
    import sys
    if '/opt/trn_rl_repo' not in sys.path:
        sys.path.insert(0, '/opt/trn_rl_repo')
    import concourse.bacc as bacc
    import concourse.bass as bass
    import concourse.mybir as mybir
    from concourse.tile import TileContext

    dt = mybir.dt.float16 if DT_POOL == "f16" else mybir.dt.float32
    f32 = mybir.dt.float32

    nc = bacc.Bacc("TRN2", target_bir_lowering=False, num_swdge_queues=4)
    feat_h = {m: nc.dram_tensor(FEAT_KEY[m], [2, C, *MAPS[m]], f32,
                                kind="ExternalInput") for m in MAP_NAMES}
    meta_h = nc.dram_tensor("meta", [1, tot_slots], mybir.dt.int32,
                            kind="ExternalInput")
    afac_h = nc.dram_tensor("afac", [128, totch * 14], dt, kind="ExternalInput")
    ident_h = nc.dram_tensor("ident", [128, 128], dt, kind="ExternalInput")
    wtil_h = nc.dram_tensor("wtil", [128, 3 * 49 * 26], dt, kind="ExternalInput")
    out_h = nc.dram_tensor("outv", [tot_slots, 26], f32, kind="ExternalOutput")

    smax = max(plan[m]["S"] for m in MAP_NAMES)

    with TileContext(nc) as tc:
        with tc.tile_pool(name="const", bufs=1) as constp, \
             tc.tile_pool(name="reg", bufs=2) as regp, \
             tc.tile_pool(name="regT", bufs=2) as regTp, \
             tc.tile_pool(name="afac", bufs=2) as afacp, \
             tc.tile_pool(name="abuf", bufs=2) as abufp, \
             tc.tile_pool(name="pooled", bufs=1) as pooledp, \
             tc.tile_pool(name="osb", bufs=2) as osbp, \
             tc.tile_pool(name="psT", bufs=3, space="PSUM") as psTp, \
             tc.tile_pool(name="psP", bufs=2, space="PSUM") as psPp, \
             tc.tile_pool(name="psG", bufs=2, space="PSUM") as psGp:

            meta_t = constp.tile([1, tot_slots], mybir.dt.int32)
            nc.sync.dma_start(meta_t[:], meta_h[:])
            id_t = constp.tile([128, 128], dt)
            nc.sync.dma_start(id_t[:], ident_h[:])
            wtil_t = constp.tile([128, 3 * 49 * 26], dt)
            nc.sync.dma_start(wtil_t[:], wtil_h[:])

            regs = [nc.gpsimd.alloc_register(f"roff{i}") for i in range(4)]

            slot_base = 0   # global slot index (meta/out row)
            ch_base = 0     # global chunk counter (afac columns)
            for mi, m in enumerate(MAP_NAMES):
                pm = plan[m]
                H, W = pm["H"], pm["W"]
                S = pm["S"]
                feat = feat_h[m]
                pooled_buf = pooledp.tile([128, 49 * smax], dt, tag="pooled")
                sl = 0  # slot index within map
                for (hc, wc, nch, count) in pm["classes"]:
                    npix = hc * wc
                    for g0 in range(0, count, GROUP):
                        gcount = min(GROUP, count - g0)
                        # afac block + A build for the group
                        af_t = afacp.tile([128, gcount * nch * 14], dt, tag="afac")
                        nc.sync.dma_start(
                            af_t[:],
                            afac_h[:, ch_base * 14:(ch_base + gcount * nch) * 14])
                        a_buf = abufp.tile([128, gcount * nch * 49], dt, tag="abuf")
                        af_ap = af_t[:]
                        pstride = af_ap.ap[0][0]
                        ay_ap = bass.AP(af_ap.tensor, af_ap.offset,
                                        [[pstride, 128], [14, gcount * nch],
                                         [1, 7], [0, 7]])
                        ax_ap = bass.AP(af_ap.tensor, af_ap.offset + 7,
                                        [[pstride, 128], [14, gcount * nch],
                                         [0, 7], [1, 7]])
                        nc.vector.tensor_tensor(out=a_buf[:], in0=ay_ap,
                                                in1=ax_ap,
                                                op=mybir.AluOpType.mult)
                        for gl in range(gcount):
                            slot = sl + g0 + gl
                            gslot = slot_base + slot
                            reg = regs[gslot % len(regs)]
                            nc.gpsimd.reg_load(reg, meta_t[0:1, gslot:gslot + 1])
                            rv = bass.RuntimeValue(reg)
                            src_ap = bass.AP(feat, rv,
                                             [[H * W, 128], [W, hc], [1, wc]])
                            region = regp.tile([128, npix], dt, tag=f"reg_{m}")
                            nc.gpsimd.dma_start(region[:], src_ap)
                            regionT = regTp.tile([128, nch * 128], dt,
                                                 tag=f"regT_{m}")
                            for c0 in range(0, nch, 4):
                                cn = min(4, nch - c0)
                                psT = psTp.tile([128, 512], dt, tag="psT")
                                for k in range(c0, c0 + cn):
                                    kk = min(128, npix - k * 128)
                                    nc.tensor.transpose(
                                        psT[0:kk, (k - c0) * 128:(k - c0 + 1) * 128],
                                        region[:, k * 128:k * 128 + kk],
                                        id_t[:])
                                nc.scalar.copy(
                                    regionT[:, c0 * 128:(c0 + cn) * 128],
                                    psT[:, 0:cn * 128])
                            pooled_ps = psPp.tile([128, 49], f32, tag="psP")
                            for k in range(nch):
                                kk = min(128, npix - k * 128)
                                nc.tensor.matmul(
                                    pooled_ps[:],
                                    lhsT=regionT[0:kk, k * 128:(k + 1) * 128],
                                    rhs=a_buf[0:kk,
                                              (gl * nch + k) * 49:
                                              (gl * nch + k + 1) * 49],
                                    start=(k == 0), stop=(k == nch - 1))
                            nc.scalar.copy(
                                pooled_buf[:, slot * 49:(slot + 1) * 49],
                                pooled_ps[:])
                        ch_base += gcount * nch
                    sl += count
                # GEMM for this map
                for b0 in range(0, S, GEMM_M):
                    mcount = min(GEMM_M, S - b0)
                    out_ps = psGp.tile([mcount, 26], f32, tag="psG")
                    pb_ap = pooled_buf[:]
                    pstride = pb_ap.ap[0][0]
                    for bin_ in range(49):
                        lhsT = bass.AP(pb_ap.tensor,
                                       pb_ap.offset + b0 * 49 + bin_,
                                       [[pstride, 128], [49, mcount]])
                        nc.tensor.matmul(
                            out_ps[:], lhsT=lhsT,
                            rhs=wtil_t[:, (mi * 49 + bin_) * 26:
                                       (mi * 49 + bin_ + 1) * 26],
                            start=(bin_ == 0), stop=(bin_ == 48))
                    out_sb = osbp.tile([mcount, 26], f32, tag="osb")
                    nc.scalar.copy(out_sb[:], out_ps[:])
                    nc.sync.dma_start(
                        out_h[slot_base + b0:slot_base + b0 + mcount, :],
                        out_sb[:])
                slot_base += S
    nc.compile()
    return nc


def kernel(**inputs):
    import sys
    if '/opt/trn_rl_repo' not in sys.path:
        sys.path.insert(0, '/opt/trn_rl_repo')
    from concourse import bass_utils

    plan = _build_plan(inputs)
    mats, cst = _collapse_head(inputs)
    wtil = _pack_wtil(mats)
    np_dt = np.float16 if DT_POOL == "f16" else np.float32
    ident = np.eye(128, dtype=np_dt)

    in_maps = []
    tot_slots = sum(plan[m]["S"] for m in MAP_NAMES)
    totch = None
    for core in range(N_CORES):
        meta, afac = _pack_core(plan, core)
        assert meta.shape[1] == tot_slots
        if totch is None:
            totch = afac.shape[1] // 14
        else:
            assert totch == afac.shape[1] // 14
        in_maps.append({
            "feat_bev": _f32(inputs["feat_bev"]),
            "feat_fv": _f32(inputs["feat_fv"]),
            "feat_rgb": _f32(inputs["feat_rgb"]),
            "meta": meta, "afac": afac, "ident": ident, "wtil": wtil,
        })

    nc = _trace_program(plan, tot_slots, totch)
    import os as _os, time as _time
    _os.environ.setdefault("BASS_NEVER_TRACE", "1")
    _t0 = _time.time()
    res = bass_utils.run_bass_kernel_spmd(nc, in_maps,
                                          core_ids=list(range(N_CORES)))
    global LAST_EXEC_TIME_NS, LAST_RUN_WALL_NS
    LAST_RUN_WALL_NS = int((_time.time() - _t0) * 1e9)
    LAST_EXEC_TIME_NS = getattr(res, "exec_time_ns", None)

    N = inputs["proposals3d"].shape[0]
    scale = WSCALE if DT_POOL == "f16" else 1.0
    tot = np.tile(cst[None, :], (N, 1))
    for core in range(N_CORES):
        outv = np.asarray(res.results[core]["outv"], np.float64) / scale
        base = 0
        for m in MAP_NAMES:
            pm = plan[m]
            slots = pm["core_slots"][core]
            for slot, r in enumerate(slots):
                if r >= 0:
                    tot[r] += outv[base + slot]
            base += pm["S"]
    cls_scores = tot[:, :2].astype(np.float32)
    bbox_pred = tot[:, 2:].astype(np.float32)
    return (cls_scores, bbox_pred)


# revision 11
# speedup vs baseline: 1.5068x; 1.5068x over previous
"""RegionFusionNetwork Trainium2 kernel.

Strategy (8 NeuronCores, SPMD single program):
- ROI-Align is expressed per ROI as pooled[c,bin] = sum_pix region[c,pix] *
  (Ay (x) Ax)[pix,bin]  -- a Kronecker-factored interpolation matrix applied
  with the tensor engine after an on-chip region transpose.
- The whole post-pooling network is linear, so the three 6272->512 FCs, the
  fusion FC and both heads collapse on the host into one [26, 6272] matrix per
  feature map; the device GEMM contracts pooled features directly to the
  26 output columns (cls 2 + reg 24).
- Per-ROI bbox regions are DMA'd channels-first [c=128, Hc, Wc] with
  register-driven dynamic base offsets; shapes are made static by bucketing
  ROIs into size classes, with identical class counts on every core (host
  deals ROIs round-robin within each class, padding with dummy slots).
- Data-parallel over the 2000 proposals; feature maps + weights replicated.
"""
import numpy as np

OUT = 7
SR = 2
P14 = OUT * SR
C = 128
N_CORES = 8
MAPS = {"bev": (800, 704), "fv": (64, 512), "rgb": (128, 512)}
MAP_NAMES = ["bev", "fv", "rgb"]
FEAT_KEY = {"bev": "feat_bev", "fv": "feat_fv", "rgb": "feat_rgb"}
GROUP = 8          # slots per A-build batch
GEMM_M = 64        # slots per GEMM output batch
DT_POOL = "f16"    # pooling-path dtype: "f16" or "f32"
WSCALE = 1024.0    # head-weight scale (undone on host) to avoid fp16 subnormals

LAST_EXEC_TIME_NS = None  # set by kernel() when a HW profile is available
LAST_RUN_WALL_NS = None   # wall time of the execute call (incl. transfers)

_GRID = [2, 3, 4, 5, 6, 7, 8, 9, 10, 11, 12, 13, 14, 15, 16, 17, 18, 19, 20,
         21, 22, 23, 24, 26, 28, 30, 33, 36, 39, 42, 46, 50, 55, 60, 66, 72,
         79, 87, 95, 104]


def _f32(x):
    return np.asarray(x, dtype=np.float32)


def _grid_up(v, cap):
    for g in _GRID:
        if g >= v:
            return min(g, cap)
    return cap


def _project_bev(p):
    b = p[:, 0]
    x_img = (p[:, 1] - np.float32(0.0)) / np.float32(0.1)
    y_img = (p[:, 2] - np.float32(-40.0)) / np.float32(0.1)
    l_img = p[:, 4] / np.float32(0.1)
    w_img = p[:, 5] / np.float32(0.1)
    hl = l_img / np.float32(2)
    hw = w_img / np.float32(2)
    return np.stack([b, x_img - hl, y_img - hw, x_img + hl, y_img + hw],
                    axis=1).astype(np.float32)


def _sample_geometry(rois, H, W):
    x1, y1, x2, y2 = rois[:, 1], rois[:, 2], rois[:, 3], rois[:, 4]
    roi_w = np.maximum(x2 - x1, np.float32(1.0))
    roi_h = np.maximum(y2 - y1, np.float32(1.0))
    frac = (np.arange(P14, dtype=np.float32) + np.float32(0.5)) / np.float32(SR)
    ty = (roi_h / np.float32(OUT))[:, None]
    tx = (roi_w / np.float32(OUT))[:, None]
    gy = y1[:, None] + frac[None, :] * ty
    gx = x1[:, None] + frac[None, :] * tx
    vy = (gy >= np.float32(-1.0)) & (gy <= np.float32(H))
    vx = (gx >= np.float32(-1.0)) & (gx <= np.float32(W))
    y = np.clip(gy, np.float32(0.0), np.float32(H - 1))
    x = np.clip(gx, np.float32(0.0), np.float32(W - 1))
    y0 = np.floor(y).astype(np.int32)
    x0 = np.floor(x).astype(np.int32)
    y1i = np.minimum(y0 + 1, H - 1)
    x1i = np.minimum(x0 + 1, W - 1)
    ly = y - y0.astype(np.float32)
    lx = x - x0.astype(np.float32)
    return dict(y0=y0, y1i=y1i, x0=x0, x1i=x1i, ly=ly, lx=lx, vy=vy, vx=vx,
                by0=y0.min(1), by1=y1i.max(1), bx0=x0.min(1), bx1=x1i.max(1))


def _axis_factor(idx0, idx1, lo, valid, origin, size):
    """A-axis factor [size, 7]: 0.5*(1-lo) at idx0, 0.5*lo at idx1 per sample."""
    A = np.zeros((size, OUT), np.float32)
    half = np.float32(0.5)
    for p in range(P14):
        if not valid[p]:
            continue
        i = p // SR
        w1 = lo[p]
        A[idx0[p] - origin, i] += half * (np.float32(1.0) - w1)
        A[idx1[p] - origin, i] += half * w1
    return A


def _plan_map(rois, H, W, bidx):
    """Per-map plan: classes, per-core slots, per-ROI origin + factors."""
    N = rois.shape[0]
    g = _sample_geometry(rois, H, W)
    Hr = g["by1"] - g["by0"] + 1
    Wr = g["bx1"] - g["bx0"] + 1
    Hc = np.array([_grid_up(int(h), H) for h in Hr], np.int32)
    Wc = np.array([_grid_up(int(w), W) for w in Wr], np.int32)
    oy = np.minimum(g["by0"], H - Hc)
    ox = np.minimum(g["bx0"], W - Wc)
    cls_of = {}
    for r in range(N):
        cls_of.setdefault((int(Hc[r]), int(Wc[r])), []).append(r)
    classes = sorted(cls_of.keys())
    # per-core slot lists, identical structure across cores
    core_slots = [[] for _ in range(N_CORES)]
    slot_classes = []   # (Hc, Wc, nch, count_per_core) per class block
    for key in classes:
        rlist = cls_of[key]
        per_core = (len(rlist) + N_CORES - 1) // N_CORES
        nch = (key[0] * key[1] + 127) // 128
        slot_classes.append((key[0], key[1], nch, per_core))
        for i in range(per_core * N_CORES):
            core_slots[i % N_CORES].append(rlist[i] if i < len(rlist) else -1)
    S = len(core_slots[0])
    return dict(g=g, oy=oy, ox=ox, Hc=Hc, Wc=Wc, classes=slot_classes,
                core_slots=core_slots, S=S, H=H, W=W, bidx=bidx)


def _build_plan(inputs):
    prop = _f32(inputs["proposals3d"])
    bidx = prop[:, 0].astype(np.int32)
    rois = {
        "bev": _project_bev(prop),
        "fv": np.concatenate([prop[:, 0:1],
                              _f32(inputs["rand_fv"]) * np.float32(50.0)],
                             axis=1).astype(np.float32),
        "rgb": np.concatenate([prop[:, 0:1],
                               _f32(inputs["rand_rgb"]) * np.float32(100.0)],
                              axis=1).astype(np.float32),
    }
    plan = {}
    for m in MAP_NAMES:
        H, W = MAPS[m]
        plan[m] = _plan_map(rois[m], H, W, bidx)
    return plan


def _pack_core(plan, core):
    """meta offsets [TOT], afac [128, TOTCH*14] for one core."""
    np_dt = np.float16 if DT_POOL == "f16" else np.float32
    metas = []
    afac_cols = []
    for m in MAP_NAMES:
        pm = plan[m]
        H, W = pm["H"], pm["W"]
        g = pm["g"]
        slots = pm["core_slots"][core]
        si = 0
        for (hc, wc, nch, count) in pm["classes"]:
            for _ in range(count):
                r = slots[si]
                si += 1
                blk = np.zeros((128, nch * 14), np.float32)
                if r < 0:
                    metas.append(0)
                else:
                    o_y, o_x = int(pm["oy"][r]), int(pm["ox"][r])
                    metas.append(int(pm["bidx"][r]) * C * H * W + o_y * W + o_x)
                    Ay = _axis_factor(g["y0"][r], g["y1i"][r], g["ly"][r],
                                      g["vy"][r], o_y, hc)
                    Ax = _axis_factor(g["x0"][r], g["x1i"][r], g["lx"][r],
                                      g["vx"][r], o_x, wc)
                    npix = hc * wc
                    t = np.arange(nch * 128)
                    ok = t < npix
                    ayr = np.zeros((nch * 128, OUT), np.float32)
                    axr = np.zeros((nch * 128, OUT), np.float32)
                    ayr[ok] = Ay[t[ok] // wc]
                    axr[ok] = Ax[t[ok] % wc]
                    blk[:, :] = np.concatenate(
                        [ayr.reshape(nch, 128, OUT), axr.reshape(nch, 128, OUT)],
                        axis=2).transpose(1, 0, 2).reshape(128, nch * 14)
                afac_cols.append(blk)
    afac = np.concatenate(afac_cols, axis=1).astype(np_dt)
    return np.asarray(metas, np.int32).reshape(1, -1), afac


def _collapse_head(inputs):
    W_fus = np.float64(1.0) * inputs["W_fus"]
    mats = {}
    for m, wk in (("bev", "W_bev"), ("fv", "W_fv"), ("rgb", "W_rgb")):
        core = np.concatenate([inputs["W_cls"], inputs["W_reg"]], axis=0)
        M = (core.astype(np.float64) @ W_fus.astype(np.float64)
             @ inputs[wk].astype(np.float64)) / 3.0
        mats[m] = M  # [26, 6272]
    b_bar = (inputs["b_bev"] + inputs["b_fv"] + inputs["b_rgb"]).astype(np.float64) / 3.0
    t = inputs["b_fus"].astype(np.float64) + inputs["W_fus"].astype(np.float64) @ b_bar
    cst = np.concatenate([
        inputs["b_cls"].astype(np.float64) + inputs["W_cls"].astype(np.float64) @ t,
        inputs["b_reg"].astype(np.float64) + inputs["W_reg"].astype(np.float64) @ t,
    ])
    return mats, cst


def _pack_wtil(mats):
    """wtil [128, 3*49*26]: wtil[c, (m*49+bin)*26+o] = M_m[o, c*49+bin]*WSCALE."""
    np_dt = np.float16 if DT_POOL == "f16" else np.float32
    scale = WSCALE if DT_POOL == "f16" else 1.0
    w = np.zeros((128, 3 * 49 * 26), np.float64)
    for mi, m in enumerate(MAP_NAMES):
        M = mats[m].reshape(26, 128, 49)  # [o, c, bin]
        w[:, (mi * 49) * 26:(mi * 49 + 49) * 26] = (
            M.transpose(1, 2, 0).reshape(128, 49 * 26) * scale)
    return w.astype(np_dt)


def _trace_program(plan, tot_slots, totch):
    import sys
    if '/opt/trn_rl_repo' not in sys.path:
        sys.path.insert(0, '/opt/trn_rl_repo')
    import concourse.bacc as bacc
    import concourse.bass as bass
    import concourse.mybir as mybir
    from concourse.tile import TileContext

    dt = mybir.dt.float16 if DT_POOL == "f16" else mybir.dt.float32
    f32 = mybir.dt.float32

    nc = bacc.Bacc("TRN2", target_bir_lowering=False, num_swdge_queues=4)
    feat_h = {m: nc.dram_tensor(FEAT_KEY[m], [2, C, *MAPS[m]], f32,
                                kind="ExternalInput") for m in MAP_NAMES}
    meta_h = nc.dram_tensor("meta", [1, tot_slots], mybir.dt.int32,
                            kind="ExternalInput")
    afac_h = nc.dram_tensor("afac", [128, totch * 14], dt, kind="ExternalInput")
    ident_h = nc.dram_tensor("ident", [128, 128], f32, kind="ExternalInput")
    ident2_h = nc.dram_tensor("ident16", [128, 128], dt, kind="ExternalInput")
    wtil_h = nc.dram_tensor("wtil", [128, 3 * 49 * 26], dt, kind="ExternalInput")
    out_h = nc.dram_tensor("outv", [tot_slots, 26], f32, kind="ExternalOutput")

    smax = max(plan[m]["S"] for m in MAP_NAMES)

    with TileContext(nc) as tc:
        with tc.tile_pool(name="const", bufs=1) as constp, \
             tc.tile_pool(name="reg", bufs=2) as regp, \
             tc.tile_pool(name="regT", bufs=2) as regTp, \
             tc.tile_pool(name="afac", bufs=2) as afacp, \
             tc.tile_pool(name="abuf", bufs=2) as abufp, \
             tc.tile_pool(name="pooled", bufs=1) as pooledp, \
             tc.tile_pool(name="osb", bufs=2) as osbp, \
             tc.tile_pool(name="psT", bufs=3, space="PSUM") as psTp, \
             tc.tile_pool(name="psP", bufs=2, space="PSUM") as psPp, \
             tc.tile_pool(name="psG", bufs=2, space="PSUM") as psGp:

            meta_t = constp.tile([1, tot_slots], mybir.dt.int32)
            nc.sync.dma_start(meta_t[:], meta_h[:])
            id_f32 = constp.tile([128, 128], f32)
            nc.sync.dma_start(id_f32[:], ident_h[:])
            id_dt = constp.tile([128, 128], dt)
            nc.sync.dma_start(id_dt[:], ident2_h[:])
            wtil_t = constp.tile([128, 3 * 49 * 26], dt)
            nc.sync.dma_start(wtil_t[:], wtil_h[:])

            regs_pool = [nc.gpsimd.alloc_register(f"roff{i}") for i in range(4)]
            regs_sp = [nc.sync.alloc_register(f"soff{i}") for i in range(4)]

            slot_base = 0   # global slot index (meta/out row)
            ch_base = 0     # global chunk counter (afac columns)
            for mi, m in enumerate(MAP_NAMES):
                pm = plan[m]
                H, W = pm["H"], pm["W"]
                S = pm["S"]
                feat = feat_h[m]
                # bev/fv regions load via HWDGE (SP sequencer) in fp32 and are
                # cast to the pooling dtype at the PSUM->SBUF evacuation; rgb
                # stays on SWDGE (Pool) with an in-DMA fp32->fp16 cast.  This
                # splits DMA issue across two sequencers.
                use_hwdge = m in ("bev", "fv")
                dma_eng = nc.sync if use_hwdge else nc.gpsimd
                regs = regs_sp if use_hwdge else regs_pool
                rdt = f32 if use_hwdge else dt
                id_t = id_f32 if use_hwdge else id_dt
                pooled_buf = pooledp.tile([128, 49 * smax], dt, tag="pooled")
                sl = 0  # slot index within map
                for (hc, wc, nch, count) in pm["classes"]:
                    npix = hc * wc
                    for g0 in range(0, count, GROUP):
                        gcount = min(GROUP, count - g0)
                        # afac block + A build for the group
                        af_t = afacp.tile([128, gcount * nch * 14], dt, tag="afac")
                        nc.sync.dma_start(
                            af_t[:],
                            afac_h[:, ch_base * 14:(ch_base + gcount * nch) * 14])
                        a_buf = abufp.tile([128, gcount * nch * 49], dt, tag="abuf")
                        af_ap = af_t[:]
                        pstride = af_ap.ap[0][0]
                        ay_ap = bass.AP(af_ap.tensor, af_ap.offset,
                                        [[pstride, 128], [14, gcount * nch],
                                         [1, 7], [0, 7]])
                        ax_ap = bass.AP(af_ap.tensor, af_ap.offset + 7,
                                        [[pstride, 128], [14, gcount * nch],
                                         [0, 7], [1, 7]])
                        nc.vector.tensor_tensor(out=a_buf[:], in0=ay_ap,
                                                in1=ax_ap,
                                                op=mybir.AluOpType.mult)
                        for gl in range(gcount):
                            slot = sl + g0 + gl
                            gslot = slot_base + slot
                            reg = regs[gslot % len(regs)]
                            dma_eng.reg_load(reg, meta_t[0:1, gslot:gslot + 1])
                            rv = bass.RuntimeValue(reg)
                            src_ap = bass.AP(feat, rv,
                                             [[H * W, 128], [W, hc], [1, wc]])
                            region = regp.tile([128, npix], rdt, tag=f"reg_{m}")
                            dma_eng.dma_start(region[:], src_ap)
                            regionT = regTp.tile([128, nch * 128], dt,
                                                 tag=f"regT_{m}")
                            for c0 in range(0, nch, 4):
                                cn = min(4, nch - c0)
                                psT = psTp.tile([128, 512], rdt, tag="psT")
                                for k in range(c0, c0 + cn):
                                    kk = min(128, npix - k * 128)
                                    nc.tensor.transpose(
                                        psT[0:kk, (k - c0) * 128:(k - c0 + 1) * 128],
                                        region[:, k * 128:k * 128 + kk],
                                        id_t[:])
                                nc.scalar.copy(
                                    regionT[:, c0 * 128:(c0 + cn) * 128],
                                    psT[:, 0:cn * 128])
                            pooled_ps = psPp.tile([128, 49], f32, tag="psP")
                            for k in range(nch):
                                kk = min(128, npix - k * 128)
                                nc.tensor.matmul(
                                    pooled_ps[:],
                                    lhsT=regionT[0:kk, k * 128:(k + 1) * 128],
                                    rhs=a_buf[0:kk,
                                              (gl * nch + k) * 49:
                                              (gl * nch + k + 1) * 49],
                                    start=(k == 0), stop=(k == nch - 1))
                            nc.scalar.copy(
                                pooled_buf[:, slot * 49:(slot + 1) * 49],
                                pooled_ps[:])
                        ch_base += gcount * nch
                    sl += count
                # GEMM for this map
                for b0 in range(0, S, GEMM_M):
                    mcount = min(GEMM_M, S - b0)
                    out_ps = psGp.tile([mcount, 26], f32, tag="psG")
                    pb_ap = pooled_buf[:]
                    pstride = pb_ap.ap[0][0]
                    for bin_ in range(49):
                        lhsT = bass.AP(pb_ap.tensor,
                                       pb_ap.offset + b0 * 49 + bin_,
                                       [[pstride, 128], [49, mcount]])
                        nc.tensor.matmul(
                            out_ps[:], lhsT=lhsT,
                            rhs=wtil_t[:, (mi * 49 + bin_) * 26:
                                       (mi * 49 + bin_ + 1) * 26],
                            start=(bin_ == 0), stop=(bin_ == 48))
                    out_sb = osbp.tile([mcount, 26], f32, tag="osb")
                    nc.scalar.copy(out_sb[:], out_ps[:])
                    nc.sync.dma_start(
                        out_h[slot_base + b0:slot_base + b0 + mcount, :],
                        out_sb[:])
                slot_base += S
    nc.compile()
    return nc


def kernel(**inputs):
    import sys
    if '/opt/trn_rl_repo' not in sys.path:
        sys.path.insert(0, '/opt/trn_rl_repo')
    from concourse import bass_utils

    plan = _build_plan(inputs)
    mats, cst = _collapse_head(inputs)
    wtil = _pack_wtil(mats)
    np_dt = np.float16 if DT_POOL == "f16" else np.float32
    ident = np.eye(128, dtype=np.float32)
    ident16 = np.eye(128, dtype=np_dt)

    in_maps = []
    tot_slots = sum(plan[m]["S"] for m in MAP_NAMES)
    totch = None
    for core in range(N_CORES):
        meta, afac = _pack_core(plan, core)
        assert meta.shape[1] == tot_slots
        if totch is None:
            totch = afac.shape[1] // 14
        else:
            assert totch == afac.shape[1] // 14
        in_maps.append({
            "feat_bev": _f32(inputs["feat_bev"]),
            "feat_fv": _f32(inputs["feat_fv"]),
            "feat_rgb": _f32(inputs["feat_rgb"]),
            "meta": meta, "afac": afac, "ident": ident, "ident16": ident16,
            "wtil": wtil,
        })

    nc = _trace_program(plan, tot_slots, totch)
    import os as _os, time as _time
    _os.environ.setdefault("BASS_NEVER_TRACE", "1")
    _t0 = _time.time()
    res = bass_utils.run_bass_kernel_spmd(nc, in_maps,
                                          core_ids=list(range(N_CORES)))
    global LAST_EXEC_TIME_NS, LAST_RUN_WALL_NS
    LAST_RUN_WALL_NS = int((_time.time() - _t0) * 1e9)
    LAST_EXEC_TIME_NS = getattr(res, "exec_time_ns", None)

    N = inputs["proposals3d"].shape[0]
    scale = WSCALE if DT_POOL == "f16" else 1.0
    tot = np.tile(cst[None, :], (N, 1))
    for core in range(N_CORES):
        outv = np.asarray(res.results[core]["outv"], np.float64) / scale
        base = 0
        for m in MAP_NAMES:
            pm = plan[m]
            slots = pm["core_slots"][core]
            for slot, r in enumerate(slots):
                if r >= 0:
                    tot[r] += outv[base + slot]
            base += pm["S"]
    cls_scores = tot[:, :2].astype(np.float32)
    bbox_pred = tot[:, 2:].astype(np.float32)
    return (cls_scores, bbox_pred)
